# revision 11
# baseline (speedup 1.0000x reference)
"""EntityGuidedCrossAttention TRN2 kernel (8 NeuronCores, data-parallel over classes).

Math restructure (exact): with labels contiguous per class and the attention
masked to each class's own support rows, the score matrix is block-diagonal:
    scores[c, k] = ((ent Wq^T + bq) Wk)[c] . sup[c*K+k] / sqrt(D)
      (the bk term is constant within a softmax row -> shift-invariant, dropped)
    attended[c]  = (sum_k w[c,k] sup[c*K+k]) Wv^T + bv   (since sum_k w = 1)
so the two big [NK,D]x[D,D] projections (Kp, Vp) are never materialized; the
kernel is memory-bound (reads sup once, writes out once, reads 4 DxD weights).

Device pipeline per core (64 classes / 4096 rows):
  A:   Qk = (entT.T @ WqT + bq) @ Wk              (PE f32r, weights streamed)
  B:   per 128-row tile: qkb = onehot.T @ Qk (PE), score = rowsum(sup*qkb/32)
       (fused DVE scalar_tensor_tensor with accum_out)
  C:   per 8-tile group: PE-transpose scores, softmax along free axis,
       PE-transpose weights back
  D:   wsup = w * sup (ACT per-partition scale); pooled += onehotT.T @ wsup
       (PE accumulation over all tiles)
  E:   OUT = ((pooled Wv^T + bv) Wo^T + bo)       (PE f32r)
  F:   res = sup + onehot.T @ OUT  (PE broadcast + DVE add), DMA out

float32r (tf32-ish) is used for the N=512 matmuls; operands must be produced
by a rounding instruction (ACT copy), so DMA'd tensors are staged and rounded.
"""

import numpy as np

N_CLASSES = 512
K_SHOTS = 64
D = 1024
NK = N_CLASSES * K_SHOTS
N_CORES = 8
C_LOC = N_CLASSES // N_CORES          # 64 classes per core
R_LOC = NK // N_CORES                 # 4096 support rows per core
P = 128
TILES = R_LOC // P                    # 32 row-tiles of 128
DCH = D // P                          # 8 contraction chunks
GSZ = 8                               # tiles per softmax group
GROUPS = TILES // GSZ                 # 4
CPT = P // K_SHOTS                    # 2 classes per tile
INV_SQRT_D = 1.0 / float(np.sqrt(D))
NH = D // 512                         # matmul free-dim halves

_NC_CACHE = None


def _build_nc():
    import concourse.bacc as bacc
    import concourse.tile as tile
    import concourse.mybir as mybir
    from concourse.masks import make_identity

    f32 = mybir.dt.float32
    f32r = mybir.dt.float32r
    bf16 = mybir.dt.bfloat16
    AX = mybir.AxisListType.X
    MUL = mybir.AluOpType.mult
    ADD = mybir.AluOpType.add
    EXP = mybir.ActivationFunctionType.Exp
    CPY = mybir.ActivationFunctionType.Copy

    nc = bacc.Bacc("TRN2", target_bir_lowering=False, debug=False,
                   num_devices=N_CORES)

    sup_d = nc.dram_tensor("sup", [R_LOC, D], f32, kind="ExternalInput").ap()
    entt_d = nc.dram_tensor("entt", [D, C_LOC], f32, kind="ExternalInput").ap()
    ind_d = nc.dram_tensor("ind", [C_LOC, R_LOC], f32, kind="ExternalInput").ap()
    indt_d = nc.dram_tensor("indt", [R_LOC, C_LOC], f32, kind="ExternalInput").ap()
    wqt_d = nc.dram_tensor("wqt", [D, D], f32, kind="ExternalInput").ap()
    wk_d = nc.dram_tensor("wk", [D, D], f32, kind="ExternalInput").ap()
    wvt_d = nc.dram_tensor("wvt", [D, D], f32, kind="ExternalInput").ap()
    wot_d = nc.dram_tensor("wot", [D, D], f32, kind="ExternalInput").ap()
    bq_d = nc.dram_tensor("bq", [1, D], f32, kind="ExternalInput").ap()
    bv_d = nc.dram_tensor("bv", [1, D], f32, kind="ExternalInput").ap()
    bo_d = nc.dram_tensor("bo", [1, D], f32, kind="ExternalInput").ap()
    res_d = nc.dram_tensor("res", [R_LOC, D], f32, kind="ExternalOutput").ap()

    with tile.TileContext(nc) as tc:
        with tc.tile_pool(name="const", bufs=1) as const:
            id128 = const.tile([P, P], f32)
            make_identity(nc, id128)
            ones_f = const.tile([1, C_LOC], f32)
            nc.vector.memset(ones_f, 1.0)
            ones_r = const.tile([1, C_LOC], f32r)
            nc.scalar.copy(out=ones_r, in_=ones_f)

            entt_r = const.tile([P, DCH * C_LOC], f32r)
            ind_b = const.tile([C_LOC, R_LOC], bf16)
            indt_r = const.tile([P, TILES * C_LOC], f32r)
            bq_r = const.tile([1, D], f32r)
            bv_r = const.tile([1, D], f32r)
            bo_r = const.tile([1, D], f32r)

            qt_r = const.tile([P, DCH * C_LOC], f32r)
            qk_hi = const.tile([C_LOC, D], bf16)
            qk_lo = const.tile([C_LOC, D], bf16)
            at_r = const.tile([P, DCH * C_LOC], f32r)
            out_hi = const.tile([C_LOC, D], bf16)
            out_lo = const.tile([C_LOC, D], bf16)
            pooled_sb = const.tile([C_LOC, D], f32)
            pooledt_r = const.tile([P, DCH * C_LOC], f32r)
            sup_all = const.tile([P, TILES * D], f32)

            # ------- staged loads + f32r rounding of small constants -------
            with tc.tile_pool(name="stg", bufs=2) as stg:
                et_f = stg.tile([P, DCH * C_LOC], f32, tag="stg")
                nc.sync.dma_start(
                    out=et_f.rearrange("p (ch c) -> p ch c", ch=DCH),
                    in_=entt_d.rearrange("(ch p) c -> p ch c", p=P),
                )
                nc.scalar.copy(out=entt_r, in_=et_f)
                for c4 in range(4):
                    ind_f = stg.tile([C_LOC, R_LOC // 4], f32, tag="stg",
                                     name=f"ind_f{c4}")
                    nc.sync.dma_start(
                        out=ind_f,
                        in_=ind_d[:, c4 * (R_LOC // 4):(c4 + 1) * (R_LOC // 4)],
                    )
                    nc.scalar.copy(
                        out=ind_b[:, c4 * (R_LOC // 4):(c4 + 1) * (R_LOC // 4)],
                        in_=ind_f,
                    )
                half_t = TILES // 2
                for c2 in range(2):
                    indt_f = stg.tile([P, half_t * C_LOC], f32, tag="stg",
                                      name=f"indt_f{c2}")
                    nc.sync.dma_start(
                        out=indt_f.rearrange("p (t c) -> p t c", t=half_t),
                        in_=indt_d.rearrange("(t p) c -> p t c", p=P)[
                            :, c2 * half_t:(c2 + 1) * half_t, :],
                    )
                    nc.scalar.copy(
                        out=indt_r[:, c2 * half_t * C_LOC:
                                   (c2 + 1) * half_t * C_LOC],
                        in_=indt_f,
                    )
                for bname, bdram, brnd in (("bq", bq_d, bq_r),
                                           ("bv", bv_d, bv_r),
                                           ("bo", bo_d, bo_r)):
                    b_f = stg.tile([1, D], f32, tag="stg", name=f"{bname}_f")
                    nc.sync.dma_start(out=b_f, in_=bdram)
                    nc.scalar.copy(out=brnd, in_=b_f)

            # ---------------- sup loads ------------------------------------
            for t in range(TILES):
                nc.sync.dma_start(
                    out=sup_all[:, t * D:(t + 1) * D],
                    in_=sup_d[t * P:(t + 1) * P, :],
                )

            # ---------------- Phase A: Qk = (entT.T @ WqT + bq) @ Wk -------
            with (
                nc.named_scope("phaseA"),
                tc.tile_pool(name="sbA", bufs=2) as sbA,
                tc.tile_pool(name="psA", bufs=2, space="PSUM") as psA,
            ):
                q_ps = psA.tile([C_LOC, D], f32, tag="proj")
                for ch in range(DCH):
                    wch_f = sbA.tile([P, D], f32, tag="wchf", bufs=3)
                    nc.sync.dma_start(out=wch_f,
                                      in_=wqt_d[ch * P:(ch + 1) * P, :])
                    wch = sbA.tile([P, D], f32r, tag="wch", bufs=3)
                    nc.scalar.copy(out=wch, in_=wch_f)
                    for nh in range(NH):
                        nc.tensor.matmul(
                            q_ps[:, nh * 512:(nh + 1) * 512],
                            entt_r[:, ch * C_LOC:(ch + 1) * C_LOC],
                            wch[:, nh * 512:(nh + 1) * 512],
                            start=(ch == 0), stop=False,
                        )
                for nh in range(NH):
                    nc.tensor.matmul(
                        q_ps[:, nh * 512:(nh + 1) * 512],
                        ones_r,
                        bq_r[:, nh * 512:(nh + 1) * 512],
                        start=False, stop=True,
                    )
                q_sb = sbA.tile([C_LOC, D], f32, tag="proj_sb", bufs=1)
                nc.scalar.copy(out=q_sb, in_=q_ps)
                for ch in range(DCH):
                    tp_ps = psA.tile([P, C_LOC], f32, tag="tp")
                    nc.tensor.transpose(
                        tp_ps, q_sb[:, ch * P:(ch + 1) * P],
                        id128[0:C_LOC, 0:C_LOC],
                    )
                    nc.scalar.copy(
                        out=qt_r[:, ch * C_LOC:(ch + 1) * C_LOC], in_=tp_ps
                    )
                qk_ps = psA.tile([C_LOC, D], f32, tag="proj")
                for ch in range(DCH):
                    wch_f = sbA.tile([P, D], f32, tag="wchf", bufs=3)
                    nc.sync.dma_start(out=wch_f,
                                      in_=wk_d[ch * P:(ch + 1) * P, :])
                    wch = sbA.tile([P, D], f32r, tag="wch", bufs=3)
                    nc.scalar.copy(out=wch, in_=wch_f)
                    for nh in range(NH):
                        nc.tensor.matmul(
                            qk_ps[:, nh * 512:(nh + 1) * 512],
                            qt_r[:, ch * C_LOC:(ch + 1) * C_LOC],
                            wch[:, nh * 512:(nh + 1) * 512],
                            start=(ch == 0), stop=(ch == DCH - 1),
                        )
                nc.scalar.copy(out=qk_hi, in_=qk_ps)
                nc.vector.tensor_tensor(out=qk_lo, in0=qk_ps, in1=qk_hi,
                                        op=mybir.AluOpType.subtract)

            # ------------- Phases B/C/D: scores, softmax, pooled -----------
            with (
                nc.named_scope("phaseBCD"),
                tc.tile_pool(name="sbB", bufs=2) as sbB,
                tc.tile_pool(name="psB", bufs=2, space="PSUM") as psB,
                tc.tile_pool(name="psP", bufs=1, space="PSUM") as psP,
            ):
                pooled_ps = psP.tile([C_LOC, D], f32)
                for g in range(GROUPS):
                    s8 = sbB.tile([P, GSZ], f32, tag="s8")
                    for j in range(GSZ):
                        t = g * GSZ + j
                        qkb = psB.tile([P, D], f32, tag="qkb")
                        for nh in range(NH):
                            nc.tensor.matmul(
                                qkb[:, nh * 512:(nh + 1) * 512],
                                ind_b[:, t * P:(t + 1) * P],
                                qk_hi[:, nh * 512:(nh + 1) * 512],
                                start=True, stop=False,
                            )
                            nc.tensor.matmul(
                                qkb[:, nh * 512:(nh + 1) * 512],
                                ind_b[:, t * P:(t + 1) * P],
                                qk_lo[:, nh * 512:(nh + 1) * 512],
                                start=False, stop=True,
                            )
                        prod = sbB.tile([P, D], f32, tag="prod", bufs=1)
                        nc.vector.scalar_tensor_tensor(
                            out=prod,
                            in0=sup_all[:, t * D:(t + 1) * D],
                            scalar=INV_SQRT_D,
                            in1=qkb,
                            op0=MUL,
                            op1=MUL,
                            accum_out=s8[:, j:j + 1],
                        )
                    # softmax over the 64 shots of each class (2 per tile)
                    st_ps = psB.tile([GSZ, P], f32, tag="sm")
                    nc.tensor.transpose(st_ps, s8, id128)
                    m_sb = sbB.tile([GSZ, CPT], f32, tag="m_sb")
                    nm_sb = sbB.tile([GSZ, CPT], f32, tag="nm_sb")
                    e_sb = sbB.tile([GSZ, P], f32, tag="e_sb")
                    r_sb = sbB.tile([GSZ, CPT], f32, tag="r_sb")
                    ri_sb = sbB.tile([GSZ, CPT], f32, tag="ri_sb")
                    w_sb = sbB.tile([GSZ, P], f32, tag="w_sb")
                    for h in range(CPT):
                        nc.vector.reduce_max(
                            m_sb[:, h:h + 1],
                            st_ps[:, h * K_SHOTS:(h + 1) * K_SHOTS],
                            axis=AX,
                        )
                    nc.vector.tensor_scalar_mul(nm_sb, m_sb, -1.0)
                    for h in range(CPT):
                        nc.scalar.activation(
                            out=e_sb[:, h * K_SHOTS:(h + 1) * K_SHOTS],
                            in_=st_ps[:, h * K_SHOTS:(h + 1) * K_SHOTS],
                            func=EXP,
                            bias=nm_sb[:, h:h + 1],
                            scale=1.0,
                        )
                        nc.vector.reduce_sum(
                            r_sb[:, h:h + 1],
                            e_sb[:, h * K_SHOTS:(h + 1) * K_SHOTS],
                            axis=AX,
                        )
                    nc.vector.reciprocal(ri_sb, r_sb)
                    for h in range(CPT):
                        nc.vector.tensor_scalar_mul(
                            w_sb[:, h * K_SHOTS:(h + 1) * K_SHOTS],
                            e_sb[:, h * K_SHOTS:(h + 1) * K_SHOTS],
                            ri_sb[:, h:h + 1],
                        )
                    wc_ps = psB.tile([P, GSZ], f32, tag="sm")
                    nc.tensor.transpose(wc_ps, w_sb, id128[0:GSZ, 0:GSZ])
                    wc_sb = sbB.tile([P, GSZ], f32, tag="wc_sb")
                    nc.scalar.copy(out=wc_sb, in_=wc_ps)
                    # D: pooled[c] += sum_p ind[c,p] * (w[p] * sup[p,:])
                    for j in range(GSZ):
                        t = g * GSZ + j
                        wsup = sbB.tile([P, D], f32r, tag="wsup")
                        nc.scalar.activation(
                            out=wsup,
                            in_=sup_all[:, t * D:(t + 1) * D],
                            func=CPY,
                            scale=wc_sb[:, j:j + 1],
                        )
                        for nh in range(NH):
                            nc.tensor.matmul(
                                pooled_ps[:, nh * 512:(nh + 1) * 512],
                                indt_r[:, t * C_LOC:(t + 1) * C_LOC],
                                wsup[:, nh * 512:(nh + 1) * 512],
                                start=(t == 0), stop=(t == TILES - 1),
                            )
                nc.scalar.copy(out=pooled_sb, in_=pooled_ps)

            # ---------------- Phase E: OUT = (pooled Wv^T + bv) Wo^T + bo --
            with (
                nc.named_scope("phaseE"),
                tc.tile_pool(name="sbE", bufs=2) as sbE,
                tc.tile_pool(name="psE", bufs=2, space="PSUM") as psE,
            ):
                for ch in range(DCH):
                    tp_ps = psE.tile([P, C_LOC], f32, tag="tp")
                    nc.tensor.transpose(
                        tp_ps, pooled_sb[:, ch * P:(ch + 1) * P],
                        id128[0:C_LOC, 0:C_LOC],
                    )
                    nc.scalar.copy(
                        out=pooledt_r[:, ch * C_LOC:(ch + 1) * C_LOC],
                        in_=tp_ps,
                    )
                a_ps = psE.tile([C_LOC, D], f32, tag="proj")
                for ch in range(DCH):
                    wch_f = sbE.tile([P, D], f32, tag="wchf")
                    nc.sync.dma_start(out=wch_f,
                                      in_=wvt_d[ch * P:(ch + 1) * P, :])
                    wch = sbE.tile([P, D], f32r, tag="wch")
                    nc.scalar.copy(out=wch, in_=wch_f)
                    for nh in range(NH):
                        nc.tensor.matmul(
                            a_ps[:, nh * 512:(nh + 1) * 512],
                            pooledt_r[:, ch * C_LOC:(ch + 1) * C_LOC],
                            wch[:, nh * 512:(nh + 1) * 512],
                            start=(ch == 0), stop=False,
                        )
                for nh in range(NH):
                    nc.tensor.matmul(
                        a_ps[:, nh * 512:(nh + 1) * 512],
                        ones_r,
                        bv_r[:, nh * 512:(nh + 1) * 512],
                        start=False, stop=True,
                    )
                a_sb = sbE.tile([C_LOC, D], f32, tag="proj_sb", bufs=1)
                nc.scalar.copy(out=a_sb, in_=a_ps)
                for ch in range(DCH):
                    tp_ps = psE.tile([P, C_LOC], f32, tag="tp")
                    nc.tensor.transpose(
                        tp_ps, a_sb[:, ch * P:(ch + 1) * P],
                        id128[0:C_LOC, 0:C_LOC],
                    )
                    nc.scalar.copy(
                        out=at_r[:, ch * C_LOC:(ch + 1) * C_LOC], in_=tp_ps
                    )
                o_ps = psE.tile([C_LOC, D], f32, tag="proj")
                for ch in range(DCH):
                    wch_f = sbE.tile([P, D], f32, tag="wchf")
                    nc.sync.dma_start(out=wch_f,
                                      in_=wot_d[ch * P:(ch + 1) * P, :])
                    wch = sbE.tile([P, D], f32r, tag="wch")
                    nc.scalar.copy(out=wch, in_=wch_f)
                    for nh in range(NH):
                        nc.tensor.matmul(
                            o_ps[:, nh * 512:(nh + 1) * 512],
                            at_r[:, ch * C_LOC:(ch + 1) * C_LOC],
                            wch[:, nh * 512:(nh + 1) * 512],
                            start=(ch == 0), stop=False,
                        )
                for nh in range(NH):
                    nc.tensor.matmul(
                        o_ps[:, nh * 512:(nh + 1) * 512],
                        ones_r,
                        bo_r[:, nh * 512:(nh + 1) * 512],
                        start=False, stop=True,
                    )
                nc.scalar.copy(out=out_hi, in_=o_ps)
                nc.vector.tensor_tensor(out=out_lo, in0=o_ps, in1=out_hi,
                                        op=mybir.AluOpType.subtract)

            # ---------------- Phase F: res = sup + OUT[class(row)] ---------
            with nc.named_scope("phaseF"), tc.tile_pool(
                    name="psF", bufs=2, space="PSUM") as psF:
                for t in range(TILES):
                    ob = psF.tile([P, D], f32, tag="ob")
                    for nh in range(NH):
                        nc.tensor.matmul(
                            ob[:, nh * 512:(nh + 1) * 512],
                            ind_b[:, t * P:(t + 1) * P],
                            out_hi[:, nh * 512:(nh + 1) * 512],
                            start=True, stop=False,
                        )
                        nc.tensor.matmul(
                            ob[:, nh * 512:(nh + 1) * 512],
                            ind_b[:, t * P:(t + 1) * P],
                            out_lo[:, nh * 512:(nh + 1) * 512],
                            start=False, stop=True,
                        )
                    nc.vector.tensor_tensor(
                        out=sup_all[:, t * D:(t + 1) * D],
                        in0=sup_all[:, t * D:(t + 1) * D],
                        in1=ob,
                        op=ADD,
                    )
                    nc.sync.dma_start(
                        out=res_d[t * P:(t + 1) * P, :],
                        in_=sup_all[:, t * D:(t + 1) * D],
                    )

    nc.compile()
    return nc


def _get_nc():
    global _NC_CACHE
    if _NC_CACHE is None:
        _NC_CACHE = _build_nc()
    return _NC_CACHE


def _prep_in_maps(support_features, entity_vectors, support_labels,
                  Wq, bq, Wk, bk, Wv, bv, Wo, bo):
    sup = np.ascontiguousarray(np.asarray(support_features, dtype=np.float32))
    ent = np.ascontiguousarray(np.asarray(entity_vectors, dtype=np.float32))
    labels = np.asarray(support_labels, dtype=np.int32)
    wqt = np.ascontiguousarray(np.asarray(Wq, dtype=np.float32).T)
    wk = np.ascontiguousarray(np.asarray(Wk, dtype=np.float32))
    wvt = np.ascontiguousarray(np.asarray(Wv, dtype=np.float32).T)
    wot = np.ascontiguousarray(np.asarray(Wo, dtype=np.float32).T)
    bq_ = np.asarray(bq, dtype=np.float32).reshape(1, D)
    bv_ = np.asarray(bv, dtype=np.float32).reshape(1, D)
    bo_ = np.asarray(bo, dtype=np.float32).reshape(1, D)
    # bk is dropped: it adds a per-class constant to each softmax row.

    expected = np.arange(NK, dtype=np.int32) // K_SHOTS
    assert np.array_equal(labels, expected), (
        "kernel assumes exactly K_SHOTS contiguous samples per class "
        "(labels == arange(NK)//K_SHOTS)"
    )

    in_maps = []
    for c in range(N_CORES):
        lab_loc = labels[c * R_LOC:(c + 1) * R_LOC] - c * C_LOC
        ind = (lab_loc[None, :] ==
               np.arange(C_LOC, dtype=np.int32)[:, None]).astype(np.float32)
        in_maps.append({
            "sup": np.ascontiguousarray(sup[c * R_LOC:(c + 1) * R_LOC]),
            "entt": np.ascontiguousarray(ent[c * C_LOC:(c + 1) * C_LOC].T),
            "ind": np.ascontiguousarray(ind),
            "indt": np.ascontiguousarray(ind.T),
            "wqt": wqt, "wk": wk, "wvt": wvt, "wot": wot,
            "bq": bq_, "bv": bv_, "bo": bo_,
        })
    return in_maps


def _run(in_maps, **kwargs):
    from concourse.bass_utils import run_bass_kernel_spmd
    nc = _get_nc()
    return run_bass_kernel_spmd(nc, in_maps, core_ids=list(range(N_CORES)),
                                **kwargs)


def kernel(support_features, entity_vectors, support_labels,
           Wq, bq, Wk, bk, Wv, bv, Wo, bo):
    in_maps = _prep_in_maps(support_features, entity_vectors, support_labels,
                            Wq, bq, Wk, bk, Wv, bv, Wo, bo)
    r = _run(in_maps)
    return np.concatenate([r.results[c]["res"] for c in range(N_CORES)], axis=0)


# revision 13
# speedup vs baseline: 1.2612x; 1.2612x over previous
"""EntityGuidedCrossAttention TRN2 kernel (8 NeuronCores, data-parallel over classes).

Math restructure (exact): with labels contiguous per class and the attention
masked to each class's own support rows, the score matrix is block-diagonal:
    scores[c, k] = ((ent Wq^T + bq) Wk)[c] . sup[c*K+k] / sqrt(D)
      (the bk term is constant within a softmax row -> shift-invariant, dropped)
    attended[c]  = (sum_k w[c,k] sup[c*K+k]) Wv^T + bv   (since sum_k w = 1)
so the two big [NK,D]x[D,D] projections (Kp, Vp) are never materialized; the
kernel is memory-bound (reads sup once, writes out once, reads 4 DxD weights).

Device pipeline per core (64 classes / 4096 rows):
  A:   Qk = (entT.T @ WqT + bq) @ Wk              (PE f32r, weights streamed)
  B:   per 128-row tile: qkb = onehot.T @ Qk (PE), score = rowsum(sup*qkb/32)
       (fused DVE scalar_tensor_tensor with accum_out)
  C:   per 8-tile group: PE-transpose scores, softmax along free axis,
       PE-transpose weights back
  D:   wsup = w * sup (ACT per-partition scale); pooled += onehotT.T @ wsup
       (PE accumulation over all tiles)
  E:   OUT = ((pooled Wv^T + bv) Wo^T + bo)       (PE f32r)
  F:   res = sup + onehot.T @ OUT  (PE broadcast + DVE add), DMA out

float32r (tf32-ish) is used for the N=512 matmuls; operands must be produced
by a rounding instruction (ACT copy), so DMA'd tensors are staged and rounded.
"""

import numpy as np

N_CLASSES = 512
K_SHOTS = 64
D = 1024
NK = N_CLASSES * K_SHOTS
N_CORES = 8
C_LOC = N_CLASSES // N_CORES          # 64 classes per core
R_LOC = NK // N_CORES                 # 4096 support rows per core
P = 128
TILES = R_LOC // P                    # 32 row-tiles of 128
DCH = D // P                          # 8 contraction chunks
GSZ = 8                               # tiles per softmax group
GROUPS = TILES // GSZ                 # 4
CPT = P // K_SHOTS                    # 2 classes per tile
INV_SQRT_D = 1.0 / float(np.sqrt(D))
NH = D // 512                         # matmul free-dim halves

_NC_CACHE = None


def _build_nc():
    import concourse.bacc as bacc
    import concourse.tile as tile
    import concourse.mybir as mybir
    from concourse.masks import make_identity

    f32 = mybir.dt.float32
    f32r = mybir.dt.float32r
    bf16 = mybir.dt.bfloat16
    AX = mybir.AxisListType.X
    MUL = mybir.AluOpType.mult
    ADD = mybir.AluOpType.add
    EXP = mybir.ActivationFunctionType.Exp
    CPY = mybir.ActivationFunctionType.Copy

    nc = bacc.Bacc("TRN2", target_bir_lowering=False, debug=False,
                   num_devices=N_CORES)

    sup_d = nc.dram_tensor("sup", [R_LOC, D], f32, kind="ExternalInput").ap()
    entt_d = nc.dram_tensor("entt", [D, C_LOC], f32, kind="ExternalInput").ap()
    ind_d = nc.dram_tensor("ind", [C_LOC, R_LOC], f32, kind="ExternalInput").ap()
    indt_d = nc.dram_tensor("indt", [R_LOC, C_LOC], f32, kind="ExternalInput").ap()
    wqt_d = nc.dram_tensor("wqt", [D, D], f32, kind="ExternalInput").ap()
    wk_d = nc.dram_tensor("wk", [D, D], f32, kind="ExternalInput").ap()
    wvt_d = nc.dram_tensor("wvt", [D, D], f32, kind="ExternalInput").ap()
    wot_d = nc.dram_tensor("wot", [D, D], f32, kind="ExternalInput").ap()
    bq_d = nc.dram_tensor("bq", [1, D], f32, kind="ExternalInput").ap()
    bv_d = nc.dram_tensor("bv", [1, D], f32, kind="ExternalInput").ap()
    bo_d = nc.dram_tensor("bo", [1, D], f32, kind="ExternalInput").ap()
    res_d = nc.dram_tensor("res", [R_LOC, D], f32, kind="ExternalOutput").ap()

    with tile.TileContext(nc) as tc:
        with tc.tile_pool(name="const", bufs=1) as const:
            id128 = const.tile([P, P], f32)
            make_identity(nc, id128)
            ones_f = const.tile([65, C_LOC], f32)
            nc.vector.memset(ones_f, 1.0)
            ones_r = const.tile([65, C_LOC], f32r)
            nc.scalar.copy(out=ones_r, in_=ones_f)

            entt_r = const.tile([P, DCH * C_LOC], f32r)
            ind_r = const.tile([C_LOC, R_LOC], f32r)
            indt_r = const.tile([P, TILES * C_LOC], f32r)
            b65_r = const.tile([65, D], f32r)

            qt_r = const.tile([P, DCH * C_LOC], f32r)
            qk_r = const.tile([C_LOC, D], f32r)
            at_r = const.tile([P, DCH * C_LOC], f32r)
            out_r = const.tile([C_LOC, D], f32r)
            pooled_sb = const.tile([C_LOC, D], f32)
            pooledt_r = const.tile([P, DCH * C_LOC], f32r)
            sup_all = const.tile([P, TILES * D], f32)

            # ------- staged loads + f32r rounding of small constants -------
            with tc.tile_pool(name="stg", bufs=2) as stg:
                et_f = stg.tile([P, DCH * C_LOC], f32, tag="stg")
                nc.sync.dma_start(
                    out=et_f.rearrange("p (ch c) -> p ch c", ch=DCH),
                    in_=entt_d.rearrange("(ch p) c -> p ch c", p=P),
                )
                nc.scalar.copy(out=entt_r, in_=et_f)
                for c4 in range(4):
                    ind_f = stg.tile([C_LOC, R_LOC // 4], f32, tag="stg",
                                     name=f"ind_f{c4}")
                    nc.sync.dma_start(
                        out=ind_f,
                        in_=ind_d[:, c4 * (R_LOC // 4):(c4 + 1) * (R_LOC // 4)],
                    )
                    nc.scalar.copy(
                        out=ind_r[:, c4 * (R_LOC // 4):(c4 + 1) * (R_LOC // 4)],
                        in_=ind_f,
                    )
                half_t = TILES // 2
                for c2 in range(2):
                    indt_f = stg.tile([P, half_t * C_LOC], f32, tag="stg",
                                      name=f"indt_f{c2}")
                    nc.sync.dma_start(
                        out=indt_f.rearrange("p (t c) -> p t c", t=half_t),
                        in_=indt_d.rearrange("(t p) c -> p t c", p=P)[
                            :, c2 * half_t:(c2 + 1) * half_t, :],
                    )
                    nc.scalar.copy(
                        out=indt_r[:, c2 * half_t * C_LOC:
                                   (c2 + 1) * half_t * C_LOC],
                        in_=indt_f,
                    )
                b65_f = stg.tile([65, D], f32, tag="stg")
                nc.sync.dma_start(out=b65_f[0:1, :], in_=bq_d)
                nc.sync.dma_start(out=b65_f[32:33, :], in_=bv_d)
                nc.sync.dma_start(out=b65_f[64:65, :], in_=bo_d)
                nc.scalar.copy(out=b65_r, in_=b65_f)

            # ---------------- Phase A: Qk = (entT.T @ WqT + bq) @ Wk -------
            with (
                nc.named_scope("phaseA"),
                tc.tile_pool(name="sbA", bufs=2) as sbA,
                tc.tile_pool(name="psA", bufs=2, space="PSUM") as psA,
            ):
                q_ps = psA.tile([C_LOC, D], f32, tag="proj")
                for ch in range(DCH):
                    wch_f = sbA.tile([P, D], f32, tag="wchf", bufs=3)
                    nc.sync.dma_start(out=wch_f,
                                      in_=wqt_d[ch * P:(ch + 1) * P, :])
                    wch = sbA.tile([P, D], f32r, tag="wch", bufs=3)
                    nc.scalar.copy(out=wch, in_=wch_f)
                    for nh in range(NH):
                        nc.tensor.matmul(
                            q_ps[:, nh * 512:(nh + 1) * 512],
                            entt_r[:, ch * C_LOC:(ch + 1) * C_LOC],
                            wch[:, nh * 512:(nh + 1) * 512],
                            start=(ch == 0), stop=False,
                        )
                for nh in range(NH):
                    nc.tensor.matmul(
                        q_ps[:, nh * 512:(nh + 1) * 512],
                        ones_r[0:1, :],
                        b65_r[0:1, nh * 512:(nh + 1) * 512],
                        start=False, stop=True,
                    )
                q_sb = sbA.tile([C_LOC, D], f32, tag="proj_sb", bufs=1)
                nc.scalar.copy(out=q_sb, in_=q_ps)
                for ch in range(DCH):
                    tp_ps = psA.tile([P, C_LOC], f32, tag="tp")
                    nc.tensor.transpose(
                        tp_ps, q_sb[:, ch * P:(ch + 1) * P],
                        id128[0:C_LOC, 0:C_LOC],
                    )
                    nc.scalar.copy(
                        out=qt_r[:, ch * C_LOC:(ch + 1) * C_LOC], in_=tp_ps
                    )
                qk_ps = psA.tile([C_LOC, D], f32, tag="proj")
                for ch in range(DCH):
                    wch_f = sbA.tile([P, D], f32, tag="wchf", bufs=3)
                    nc.sync.dma_start(out=wch_f,
                                      in_=wk_d[ch * P:(ch + 1) * P, :])
                    wch = sbA.tile([P, D], f32r, tag="wch", bufs=3)
                    nc.scalar.copy(out=wch, in_=wch_f)
                    for nh in range(NH):
                        nc.tensor.matmul(
                            qk_ps[:, nh * 512:(nh + 1) * 512],
                            qt_r[:, ch * C_LOC:(ch + 1) * C_LOC],
                            wch[:, nh * 512:(nh + 1) * 512],
                            start=(ch == 0), stop=(ch == DCH - 1),
                        )
                nc.scalar.copy(out=qk_r, in_=qk_ps)

            # ---------------- sup loads ------------------------------------
            for t in range(TILES):
                nc.sync.dma_start(
                    out=sup_all[:, t * D:(t + 1) * D],
                    in_=sup_d[t * P:(t + 1) * P, :],
                )

            # ------------- Phases B/C/D: scores, softmax, pooled -----------
            with (
                nc.named_scope("phaseBCD"),
                tc.tile_pool(name="sbB", bufs=2) as sbB,
                tc.tile_pool(name="psB", bufs=2, space="PSUM") as psB,
                tc.tile_pool(name="psP", bufs=1, space="PSUM") as psP,
            ):
                pooled_ps = psP.tile([C_LOC, D], f32)
                for g in range(GROUPS):
                    s8 = sbB.tile([P, GSZ], f32, tag="s8")
                    for j in range(GSZ):
                        t = g * GSZ + j
                        qkb = psB.tile([P, D], f32, tag="qkb")
                        for nh in range(NH):
                            nc.tensor.matmul(
                                qkb[:, nh * 512:(nh + 1) * 512],
                                ind_r[:, t * P:(t + 1) * P],
                                qk_r[:, nh * 512:(nh + 1) * 512],
                                start=True, stop=True,
                            )
                        prod = sbB.tile([P, D], f32, tag="prod", bufs=1)
                        nc.vector.scalar_tensor_tensor(
                            out=prod,
                            in0=sup_all[:, t * D:(t + 1) * D],
                            scalar=INV_SQRT_D,
                            in1=qkb,
                            op0=MUL,
                            op1=MUL,
                            accum_out=s8[:, j:j + 1],
                        )
                    # softmax over the 64 shots of each class (2 per tile)
                    st_ps = psB.tile([GSZ, P], f32, tag="sm")
                    nc.tensor.transpose(st_ps, s8, id128)
                    m_sb = sbB.tile([GSZ, CPT], f32, tag="m_sb")
                    nm_sb = sbB.tile([GSZ, CPT], f32, tag="nm_sb")
                    e_sb = sbB.tile([GSZ, P], f32, tag="e_sb")
                    r_sb = sbB.tile([GSZ, CPT], f32, tag="r_sb")
                    ri_sb = sbB.tile([GSZ, CPT], f32, tag="ri_sb")
                    w_sb = sbB.tile([GSZ, P], f32, tag="w_sb")
                    for h in range(CPT):
                        nc.vector.reduce_max(
                            m_sb[:, h:h + 1],
                            st_ps[:, h * K_SHOTS:(h + 1) * K_SHOTS],
                            axis=AX,
                        )
                    nc.vector.tensor_scalar_mul(nm_sb, m_sb, -1.0)
                    for h in range(CPT):
                        nc.scalar.activation(
                            out=e_sb[:, h * K_SHOTS:(h + 1) * K_SHOTS],
                            in_=st_ps[:, h * K_SHOTS:(h + 1) * K_SHOTS],
                            func=EXP,
                            bias=nm_sb[:, h:h + 1],
                            scale=1.0,
                        )
                        nc.vector.reduce_sum(
                            r_sb[:, h:h + 1],
                            e_sb[:, h * K_SHOTS:(h + 1) * K_SHOTS],
                            axis=AX,
                        )
                    nc.vector.reciprocal(ri_sb, r_sb)
                    for h in range(CPT):
                        nc.vector.tensor_scalar_mul(
                            w_sb[:, h * K_SHOTS:(h + 1) * K_SHOTS],
                            e_sb[:, h * K_SHOTS:(h + 1) * K_SHOTS],
                            ri_sb[:, h:h + 1],
                        )
                    wc_ps = psB.tile([P, GSZ], f32, tag="sm")
                    nc.tensor.transpose(wc_ps, w_sb, id128[0:GSZ, 0:GSZ])
                    wc_sb = sbB.tile([P, GSZ], f32, tag="wc_sb")
                    nc.scalar.copy(out=wc_sb, in_=wc_ps)
                    # D: pooled[c] += sum_p ind[c,p] * (w[p] * sup[p,:])
                    for j in range(GSZ):
                        t = g * GSZ + j
                        wsup = sbB.tile([P, D], f32r, tag="wsup")
                        nc.scalar.activation(
                            out=wsup,
                            in_=sup_all[:, t * D:(t + 1) * D],
                            func=CPY,
                            scale=wc_sb[:, j:j + 1],
                        )
                        for nh in range(NH):
                            nc.tensor.matmul(
                                pooled_ps[:, nh * 512:(nh + 1) * 512],
                                indt_r[:, t * C_LOC:(t + 1) * C_LOC],
                                wsup[:, nh * 512:(nh + 1) * 512],
                                start=(t == 0), stop=(t == TILES - 1),
                            )
                nc.scalar.copy(out=pooled_sb, in_=pooled_ps)

            # ---------------- Phase E: OUT = (pooled Wv^T + bv) Wo^T + bo --
            with (
                nc.named_scope("phaseE"),
                tc.tile_pool(name="sbE", bufs=2) as sbE,
                tc.tile_pool(name="psE", bufs=2, space="PSUM") as psE,
            ):
                for ch in range(DCH):
                    tp_ps = psE.tile([P, C_LOC], f32, tag="tp")
                    nc.tensor.transpose(
                        tp_ps, pooled_sb[:, ch * P:(ch + 1) * P],
                        id128[0:C_LOC, 0:C_LOC],
                    )
                    nc.scalar.copy(
                        out=pooledt_r[:, ch * C_LOC:(ch + 1) * C_LOC],
                        in_=tp_ps,
                    )
                a_ps = psE.tile([C_LOC, D], f32, tag="proj")
                for ch in range(DCH):
                    wch_f = sbE.tile([P, D], f32, tag="wchf")
                    nc.sync.dma_start(out=wch_f,
                                      in_=wvt_d[ch * P:(ch + 1) * P, :])
                    wch = sbE.tile([P, D], f32r, tag="wch")
                    nc.scalar.copy(out=wch, in_=wch_f)
                    for nh in range(NH):
                        nc.tensor.matmul(
                            a_ps[:, nh * 512:(nh + 1) * 512],
                            pooledt_r[:, ch * C_LOC:(ch + 1) * C_LOC],
                            wch[:, nh * 512:(nh + 1) * 512],
                            start=(ch == 0), stop=False,
                        )
                for nh in range(NH):
                    nc.tensor.matmul(
                        a_ps[:, nh * 512:(nh + 1) * 512],
                        ones_r[32:33, :],
                        b65_r[32:33, nh * 512:(nh + 1) * 512],
                        start=False, stop=True,
                    )
                a_sb = sbE.tile([C_LOC, D], f32, tag="proj_sb", bufs=1)
                nc.scalar.copy(out=a_sb, in_=a_ps)
                for ch in range(DCH):
                    tp_ps = psE.tile([P, C_LOC], f32, tag="tp")
                    nc.tensor.transpose(
                        tp_ps, a_sb[:, ch * P:(ch + 1) * P],
                        id128[0:C_LOC, 0:C_LOC],
                    )
                    nc.scalar.copy(
                        out=at_r[:, ch * C_LOC:(ch + 1) * C_LOC], in_=tp_ps
                    )
                o_ps = psE.tile([C_LOC, D], f32, tag="proj")
                for ch in range(DCH):
                    wch_f = sbE.tile([P, D], f32, tag="wchf")
                    nc.sync.dma_start(out=wch_f,
                                      in_=wot_d[ch * P:(ch + 1) * P, :])
                    wch = sbE.tile([P, D], f32r, tag="wch")
                    nc.scalar.copy(out=wch, in_=wch_f)
                    for nh in range(NH):
                        nc.tensor.matmul(
                            o_ps[:, nh * 512:(nh + 1) * 512],
                            at_r[:, ch * C_LOC:(ch + 1) * C_LOC],
                            wch[:, nh * 512:(nh + 1) * 512],
                            start=(ch == 0), stop=False,
                        )
                for nh in range(NH):
                    nc.tensor.matmul(
                        o_ps[:, nh * 512:(nh + 1) * 512],
                        ones_r[64:65, :],
                        b65_r[64:65, nh * 512:(nh + 1) * 512],
                        start=False, stop=True,
                    )
                nc.scalar.copy(out=out_r, in_=o_ps)

            # ---------------- Phase F: res = sup + OUT[class(row)] ---------
            with nc.named_scope("phaseF"), tc.tile_pool(
                    name="psF", bufs=2, space="PSUM") as psF:
                for t in range(TILES):
                    ob = psF.tile([P, D], f32, tag="ob")
                    for nh in range(NH):
                        nc.tensor.matmul(
                            ob[:, nh * 512:(nh + 1) * 512],
                            ind_r[:, t * P:(t + 1) * P],
                            out_r[:, nh * 512:(nh + 1) * 512],
                            start=True, stop=True,
                        )
                    nc.vector.tensor_tensor(
                        out=sup_all[:, t * D:(t + 1) * D],
                        in0=sup_all[:, t * D:(t + 1) * D],
                        in1=ob,
                        op=ADD,
                    )
                    nc.sync.dma_start(
                        out=res_d[t * P:(t + 1) * P, :],
                        in_=sup_all[:, t * D:(t + 1) * D],
                    )

    nc.compile()
    return nc


def _get_nc():
    global _NC_CACHE
    if _NC_CACHE is None:
        _NC_CACHE = _build_nc()
    return _NC_CACHE


def _prep_in_maps(support_features, entity_vectors, support_labels,
                  Wq, bq, Wk, bk, Wv, bv, Wo, bo):
    sup = np.ascontiguousarray(np.asarray(support_features, dtype=np.float32))
    ent = np.ascontiguousarray(np.asarray(entity_vectors, dtype=np.float32))
    labels = np.asarray(support_labels, dtype=np.int32)
    wqt = np.ascontiguousarray(np.asarray(Wq, dtype=np.float32).T)
    wk = np.ascontiguousarray(np.asarray(Wk, dtype=np.float32))
    wvt = np.ascontiguousarray(np.asarray(Wv, dtype=np.float32).T)
    wot = np.ascontiguousarray(np.asarray(Wo, dtype=np.float32).T)
    bq_ = np.asarray(bq, dtype=np.float32).reshape(1, D)
    bv_ = np.asarray(bv, dtype=np.float32).reshape(1, D)
    bo_ = np.asarray(bo, dtype=np.float32).reshape(1, D)
    # bk is dropped: it adds a per-class constant to each softmax row.

    expected = np.arange(NK, dtype=np.int32) // K_SHOTS
    assert np.array_equal(labels, expected), (
        "kernel assumes exactly K_SHOTS contiguous samples per class "
        "(labels == arange(NK)//K_SHOTS)"
    )

    in_maps = []
    for c in range(N_CORES):
        lab_loc = labels[c * R_LOC:(c + 1) * R_LOC] - c * C_LOC
        ind = (lab_loc[None, :] ==
               np.arange(C_LOC, dtype=np.int32)[:, None]).astype(np.float32)
        in_maps.append({
            "sup": np.ascontiguousarray(sup[c * R_LOC:(c + 1) * R_LOC]),
            "entt": np.ascontiguousarray(ent[c * C_LOC:(c + 1) * C_LOC].T),
            "ind": np.ascontiguousarray(ind),
            "indt": np.ascontiguousarray(ind.T),
            "wqt": wqt, "wk": wk, "wvt": wvt, "wot": wot,
            "bq": bq_, "bv": bv_, "bo": bo_,
        })
    return in_maps


def _run(in_maps, **kwargs):
    from concourse.bass_utils import run_bass_kernel_spmd
    nc = _get_nc()
    return run_bass_kernel_spmd(nc, in_maps, core_ids=list(range(N_CORES)),
                                **kwargs)


def kernel(support_features, entity_vectors, support_labels,
           Wq, bq, Wk, bk, Wv, bv, Wo, bo):
    in_maps = _prep_in_maps(support_features, entity_vectors, support_labels,
                            Wq, bq, Wk, bk, Wv, bv, Wo, bo)
    r = _run(in_maps)
    return np.concatenate([r.results[c]["res"] for c in range(N_CORES)], axis=0)
